# revision 12
# baseline (speedup 1.0000x reference)
"""Multi-head attention block (QKV proj + softmax attention + out proj) on 8
Trainium2 NeuronCores, data-parallel over the batch dimension (one batch
element per core).

Self-contained: hardcodes shapes for x [8, 1024, 768], qkv_w [768, 2304],
proj_w [768, 768], proj_b [768]; returns [8, 1024, 768] float32.

v5: bf16 matmul path throughout.  K^T is stored zero-padded per head
(even heads in partitions 0:64, odd heads in 64:128, zeros elsewhere) so
the score matmuls contract K=128 against the stacked two-head Q^T — every
matmul in the kernel then runs in the 128-row tiling mode and the PE never
pays a tiling-mode-switch drain.  Attention is emitted per head-pair with
the next pair's Q/K matmuls and the V matmuls as PE filler between the
exp-gated score groups.  The projection pre-runs ct 0..4 of every token
tile (partial sums evacuated to SBUF) while head-pair 5's softmax
normalization (reciprocal via a DRAM round-trip) completes.
"""

import numpy as np

import concourse.bass as bass
import concourse.mybir as mybir
import concourse.tile as tile
from concourse import bacc

N_CORES = 8
N = 1024          # tokens per batch element
C = 768           # model dim
H = 12            # heads
HD = 64           # head dim
CT = C // 128     # 6 contraction tiles
TT = N // 128     # 8 token tiles
SCALE = HD ** -0.5

F32 = mybir.dt.float32
BF = mybir.dt.bfloat16


def _build():
    nc = bacc.Bacc("TRN2", target_bir_lowering=False, debug=False,
                   num_devices=N_CORES)
    x_t = nc.dram_tensor("x_t", [C, N], BF, kind="ExternalInput").ap()
    qkv_w = nc.dram_tensor("qkv_w", [C, 3 * C], BF, kind="ExternalInput").ap()
    proj_w = nc.dram_tensor("proj_w", [C, C], BF, kind="ExternalInput").ap()
    proj_b = nc.dram_tensor("proj_b", [1, C], F32, kind="ExternalInput").ap()
    out = nc.dram_tensor("out", [N, C], F32, kind="ExternalOutput").ap()

    with tile.TileContext(nc) as tc:
        _emit(nc, tc, x_t, qkv_w, proj_w, proj_b, out)
    nc.compile()
    return nc


def _emit(nc, tc, x_t, qkv_w, proj_w, proj_b, out):
    from contextlib import ExitStack
    ctx = ExitStack()
    with ctx:
        xt_pool = ctx.enter_context(tc.tile_pool(name="xt", bufs=1))
        w_pool = ctx.enter_context(tc.tile_pool(name="w", bufs=1))
        pw_pool = ctx.enter_context(tc.tile_pool(name="pw", bufs=1))
        qk_pool = ctx.enter_context(tc.tile_pool(name="qk", bufs=1))
        vaug_pool = ctx.enter_context(tc.tile_pool(name="vaug", bufs=1))
        ot_pool = ctx.enter_context(tc.tile_pool(name="ot", bufs=1))
        exps_pool = ctx.enter_context(tc.tile_pool(name="exps", bufs=15))
        misc_pool = ctx.enter_context(tc.tile_pool(name="misc", bufs=3))
        norm_pool = ctx.enter_context(tc.tile_pool(name="norm", bufs=2))
        const_pool = ctx.enter_context(tc.tile_pool(name="const", bufs=1))
        outsb_pool = ctx.enter_context(tc.tile_pool(name="outsb", bufs=8))
        dram_pool = ctx.enter_context(tc.tile_pool(name="drs", bufs=2, space="DRAM"))

        # ---- loads: per-ct row chunks (contiguous >=1.5KB descriptors) ----
        # Q/K weight columns + x land first so the Q^T/K^T matmuls for
        # head-pair 0 (and with them the exp stream) start early.
        XT = xt_pool.tile([128, CT, N], BF, tag="xt")
        W = w_pool.tile([128, CT, 3 * C], BF, tag="w")
        for ct in range(CT):
            nc.sync.dma_start(W[:, ct, 0:C],
                              qkv_w[ct * 128:(ct + 1) * 128, 0:C])
            nc.sync.dma_start(W[:, ct, C:2 * C],
                              qkv_w[ct * 128:(ct + 1) * 128, C:2 * C])
            nc.sync.dma_start(XT[:, ct, 0:512],
                              x_t[ct * 128:(ct + 1) * 128, 0:512])
        for ct in range(CT):
            nc.sync.dma_start(XT[:, ct, 512:1024],
                              x_t[ct * 128:(ct + 1) * 128, 512:1024])
        for ct in range(CT):
            nc.sync.dma_start(W[:, ct, 2 * C:3 * C],
                              qkv_w[ct * 128:(ct + 1) * 128, 2 * C:3 * C])
        pbb = const_pool.tile([128, C], F32, tag="pb")
        pb_src = proj_b[:, :]
        pb_bcast = bass.AP(tensor=pb_src.tensor, offset=pb_src.offset,
                           ap=[[0, 128]] + [list(a) for a in pb_src.ap[1:]])
        nc.sync.dma_start(pbb[:], pb_bcast)

        # V_AUG: [keys, kt, head, dim] with dim 64 = ones column (softmax
        # denominator rides along in the AV matmul); head 12 is a zero pad so
        # the 128-wide stationary slice for head 11 stays in bounds.
        V_AUG = vaug_pool.tile([128, TT, H + 1, HD + 1], BF, tag="vaug")
        nc.vector.memset(
            V_AUG[:].rearrange("p t h d -> p (t h d)"), 0.0)
        nc.vector.memset(
            V_AUG[:, :, :, HD:HD + 1].rearrange("p t h one -> p (t h one)"),
            1.0)
        VA_flat = V_AUG.rearrange("p t h d -> p t (h d)")

        # K^T zero-padded per head: even head in partitions 0:64, odd head
        # in 64:128, zeros elsewhere -> score matmuls contract K=128 against
        # the stacked two-head Q^T (zeros annihilate the other head) and the
        # PE stays in 128-row tiling mode for the whole kernel.
        QT = qk_pool.tile([128, CT, N], BF, tag="qt")
        KTz = qk_pool.tile([128, H, N], BF, tag="ktz")
        nc.vector.memset(KTz[:].rearrange("p h n -> p (h n)"), 0.0)

        sc_ps = ctx.enter_context(tc.tile_pool(name="scps", bufs=2, space="PSUM"))
        av_ps = ctx.enter_context(tc.tile_pool(name="avps", bufs=2, space="PSUM"))

        def emit_v(tt):
            """v = x @ Wv for one key tile, laid out per key-tile."""
            for vc, (w0, wn, h0) in enumerate([(0, 512, 0), (512, 256, 8)]):
                ps = sc_ps.tile([128, 1024], F32, tag="scps")
                for ct in range(CT):
                    nc.tensor.matmul(
                        ps[:, :wn],
                        lhsT=XT[:, ct, tt * 128:(tt + 1) * 128],
                        rhs=W[:, ct, 2 * C + w0:2 * C + w0 + wn],
                        start=(ct == 0), stop=(ct == CT - 1))
                nc.vector.tensor_copy(
                    V_AUG[:, tt, h0:h0 + wn // HD, 0:HD],
                    ps[:, :wn].rearrange("p (h d) -> p h d", d=HD))

        def emit_qk_mms(hp):
            """Thunks, each emitting one ps-group of hp's Q^T/K^T matmuls."""
            thunks = []
            for ft in (hp, CT + hp):
                for qc in range(2):
                    def emit(ft=ft, qc=qc):
                        ps = sc_ps.tile([128, 1024], F32, tag="scps")
                        for ct in range(CT):
                            nc.tensor.matmul(
                                ps[:, 0:512],
                                lhsT=W[:, ct, ft * 128:(ft + 1) * 128],
                                rhs=XT[:, ct, qc * 512:(qc + 1) * 512],
                                start=(ct == 0), stop=(ct == CT - 1))
                        if ft < CT:
                            nc.vector.tensor_copy(
                                QT[:, ft, qc * 512:(qc + 1) * 512],
                                ps[:, 0:512])
                        else:
                            fi = ft - CT
                            nc.vector.tensor_copy(
                                KTz[0:64, 2 * fi, qc * 512:(qc + 1) * 512],
                                ps[0:64, 0:512])
                            nc.vector.tensor_copy(
                                KTz[64:128, 2 * fi + 1, qc * 512:(qc + 1) * 512],
                                ps[64:128, 0:512])
                    thunks.append(emit)
            return thunks

        for th in emit_qk_mms(0):
            th()

        PW = pw_pool.tile([128, CT, C], BF, tag="pw")
        for ct in range(CT):
            nc.sync.dma_start(PW[:, ct, :], proj_w[ct * 128:(ct + 1) * 128, :])

        outT = ot_pool.tile([128, CT, N], BF, tag="ot")

        def emit_scores(hp, kt):
            psA = sc_ps.tile([128, 1024], F32, tag="scps")
            psB = sc_ps.tile([128, 1024], F32, tag="scps")
            for ps, h in ((psA, 2 * hp), (psB, 2 * hp + 1)):
                for qc in range(2):
                    nc.tensor.matmul(
                        ps[:, qc * 512:(qc + 1) * 512],
                        lhsT=KTz[:, h, kt * 128:(kt + 1) * 128],
                        rhs=QT[:, hp, qc * 512:(qc + 1) * 512],
                        start=True, stop=True)
            eA = exps_pool.tile([128, 1024], BF, tag="exps")
            eB = exps_pool.tile([128, 1024], BF, tag="exps")
            nc.scalar.activation(eA[:], psA[:], mybir.ActivationFunctionType.Exp,
                                 scale=SCALE)
            nc.scalar.activation(eB[:], psB[:], mybir.ActivationFunctionType.Exp,
                                 scale=SCALE)
            return eA, eB

        norm_jobs = []
        for hp in range(CT):
            avA = av_ps.tile([128, 1024], F32, tag="avps")
            avB = av_ps.tile([128, 1024], F32, tag="avps")
            nexts = emit_qk_mms(hp + 1) if hp + 1 < CT else []

            def emit_av(kt, eA, eB, hp=hp, avA=avA, avB=avB):
                for av, e, h in ((avA, eA, 2 * hp), (avB, eB, 2 * hp + 1)):
                    for qc in range(2):
                        nc.tensor.matmul(
                            av[:, qc * 512:(qc + 1) * 512],
                            lhsT=VA_flat[:, kt, h * 65:h * 65 + 128],
                            rhs=e[:, qc * 512:(qc + 1) * 512],
                            start=(kt == 0), stop=(kt == TT - 1))

            pend = []
            if hp == 0:
                # scores lead the PE queue (they gate the exp stream); V
                # matmuls trail each scores group as filler; AVs flush as
                # soon as their V tile exists.
                for kt in range(TT):
                    pend.append((kt, *emit_scores(0, kt)))
                    if kt >= 2:
                        emit_v(kt - 2)
                    if kt >= 6:
                        emit_av(*pend.pop(0))
                for tt in (TT - 2, TT - 1):
                    emit_v(tt)
                for args in pend:
                    emit_av(*args)
            else:
                for kt in range(TT):
                    pend.append((kt, *emit_scores(hp, kt)))
                    if len(pend) >= 3:
                        emit_av(*pend.pop(0))
                    if kt % 2 == 1 and nexts:
                        nexts.pop(0)()
                for args in pend:
                    emit_av(*args)
            pend = []
            for th in nexts:
                th()

            # evacuate AV PSUM; the denominator row goes through a DRAM
            # round-trip (reshape + partition broadcast) for the reciprocal
            last = hp == CT - 1
            UA = norm_pool.tile([64, 1024], F32, tag="U")
            UB = norm_pool.tile([64, 1024], F32, tag="U")
            UdA = misc_pool.tile([1, 1024], F32, tag="Ud")
            UdB = misc_pool.tile([1, 1024], F32, tag="Ud")
            if last:
                # scalar engine (idle by now) extracts the denominator rows
                # so both DVE row-copies and the DMA chain start immediately
                nc.scalar.copy(UdA[:], avA[HD:HD + 1, :])
                nc.scalar.copy(UdB[:], avB[HD:HD + 1, :])
                nc.vector.tensor_copy(UA[:], avA[0:HD, :])
                nc.vector.tensor_copy(UB[:], avB[0:HD, :])
            else:
                nc.vector.tensor_copy(UA[:], avA[0:HD, :])
                nc.vector.tensor_copy(UdA[:], avA[HD:HD + 1, :])
                nc.vector.tensor_copy(UB[:], avB[0:HD, :])
                nc.vector.tensor_copy(UdB[:], avB[HD:HD + 1, :])

            def emit_norm(hp=hp, pairs=((UA, UdA, 0), (UB, UdB, 64))):
                for U, Ud, poff in pairs:
                    dscr = dram_pool.tile([1024], F32, tag="dscr")
                    nc.sync.dma_start(dscr[:], Ud[0:1, :])
                    Dt = misc_pool.tile([64, 16], F32, tag="Dt")
                    nc.sync.dma_start(Dt[:], dscr[:].rearrange("(p j) -> p j", j=16))
                    Rt = misc_pool.tile([64, 16], F32, tag="Rt")
                    scr = misc_pool.tile([64, 16], F32, tag="scr")
                    nc.vector.reciprocal_approx_accurate(Rt[:], Dt[:], scr[:])
                    rscr = dram_pool.tile([1024], F32, tag="rscr")
                    nc.sync.dma_start(rscr[:].rearrange("(p j) -> p j", j=16), Rt[:])
                    for qc in range(2):
                        bc = norm_pool.tile([64, 512], F32, tag="bc")
                        rs = rscr[qc * 512:(qc + 1) * 512]
                        bcast_ap = bass.AP(tensor=rs.tensor, offset=rs.offset,
                                           ap=[[0, 64]] + [list(a) for a in rs.ap])
                        nc.sync.dma_start(bc[:], bcast_ap)
                        nc.vector.tensor_mul(
                            outT[poff:poff + 64, hp, qc * 512:(qc + 1) * 512],
                            U[:, qc * 512:(qc + 1) * 512],
                            bc[:])

            if not last:
                emit_norm()
            else:
                norm_jobs.append(emit_norm)

        # ---- proj + bias ----
        # Pre-run ct 0..4 of every token tile (partial sums parked in SBUF,
        # PSUM slots released) while head-pair 5's normalization completes;
        # only the ct=5 matmuls wait on it.
        for job in norm_jobs:
            job()
        osbs = []
        for tt in range(TT):
            osb = outsb_pool.tile([128, C], F32, tag="outsb")
            for nch in range(2):
                ps = (sc_ps if nch == 0 else av_ps).tile(
                    [128, 1024], F32, tag="scps" if nch == 0 else "avps")
                for ct in range(CT - 1):
                    nc.tensor.matmul(
                        ps[:, 0:384],
                        lhsT=outT[:, ct, tt * 128:(tt + 1) * 128],
                        rhs=PW[:, ct, nch * 384:(nch + 1) * 384],
                        start=(ct == 0), stop=(ct == CT - 2))
                nc.vector.tensor_add(osb[:, nch * 384:(nch + 1) * 384],
                                     ps[:, 0:384],
                                     pbb[:, nch * 384:(nch + 1) * 384])
            osbs.append(osb)
        for tt in range(TT):
            osb = osbs[tt]
            for nch in range(2):
                ps = (sc_ps if nch == 0 else av_ps).tile(
                    [128, 1024], F32, tag="scps" if nch == 0 else "avps")
                nc.tensor.matmul(
                    ps[:, 0:384],
                    lhsT=outT[:, CT - 1, tt * 128:(tt + 1) * 128],
                    rhs=PW[:, CT - 1, nch * 384:(nch + 1) * 384],
                    start=True, stop=True)
                nc.vector.tensor_add(osb[:, nch * 384:(nch + 1) * 384],
                                     osb[:, nch * 384:(nch + 1) * 384],
                                     ps[:, 0:384])
            # split across queues to shorten the final-DMA tail
            for q4 in range(4):
                nc.sync.dma_start(
                    out[tt * 128:(tt + 1) * 128, q4 * 192:(q4 + 1) * 192],
                    osb[:, q4 * 192:(q4 + 1) * 192])


_CACHE = {}


def _get_runner():
    """Build + compile once; return a callable(in_maps) -> list of out dicts.

    Keeps a persistent jitted shard_map executable so repeat calls skip
    retracing/recompiling (mirrors bass2jax.run_bass_via_pjrt).
    """
    if "runner" in _CACHE:
        return _CACHE["runner"]

    import jax
    from jax.experimental.shard_map import shard_map
    from jax.sharding import Mesh, PartitionSpec
    from concourse import bass2jax

    nc = _build()
    bass2jax.install_neuronx_cc_hook()

    partition_name = (nc.partition_id_tensor.name if nc.partition_id_tensor
                      else None)
    in_names, out_names, out_avals, zero_outs = [], [], [], []
    for alloc in nc.m.functions[0].allocations:
        if not isinstance(alloc, mybir.MemoryLocationSet):
            continue
        name = alloc.memorylocations[0].name
        if alloc.kind == "ExternalInput":
            if name != partition_name:
                in_names.append(name)
        elif alloc.kind == "ExternalOutput":
            out_names.append(name)
            shape = tuple(alloc.tensor_shape)
            dtype = mybir.dt.np(alloc.dtype)
            out_avals.append(jax.core.ShapedArray(shape, dtype))
            zero_outs.append(np.zeros(shape, dtype))
    n_params = len(in_names)
    n_outs = len(out_avals)
    all_in_names = list(in_names) + list(out_names)
    if partition_name is not None:
        all_in_names.append(partition_name)
    donate = tuple(range(n_params, n_params + n_outs))

    def _body(*args):
        operands = list(args)
        if partition_name is not None:
            operands.append(bass2jax.partition_id_tensor())
        outs = bass2jax._bass_exec_p.bind(
            *operands,
            out_avals=tuple(out_avals),
            in_names=tuple(all_in_names),
            out_names=tuple(out_names),
            lowering_input_output_aliases=(),
            sim_require_finite=True,
            sim_require_nnan=True,
            nc=nc,
        )
        return tuple(outs)

    devices = jax.devices()[:N_CORES]
    mesh = Mesh(np.asarray(devices), ("core",))
    in_specs = (PartitionSpec("core"),) * (n_params + n_outs)
    out_specs = (PartitionSpec("core"),) * n_outs
    sharded = jax.jit(
        shard_map(_body, mesh=mesh, in_specs=in_specs, out_specs=out_specs,
                  check_rep=False),
        donate_argnums=donate, keep_unused=True)

    def runner(in_maps):
        concat_in = [
            np.concatenate([np.asarray(m[name]) for m in in_maps], axis=0)
            for name in in_names
        ]
        concat_zeros = [
            np.zeros((N_CORES * z.shape[0], *z.shape[1:]), z.dtype)
            for z in zero_outs
        ]
        out_arrs = sharded(*concat_in, *concat_zeros)
        return [
            {name: np.asarray(out_arrs[i]).reshape(N_CORES, *out_avals[i].shape)[c]
             for i, name in enumerate(out_names)}
            for c in range(N_CORES)
        ]

    _CACHE["runner"] = runner
    _CACHE["nc"] = nc
    return runner


def make_in_maps(x, qkv_w, proj_w, proj_b):
    import ml_dtypes
    bf16 = ml_dtypes.bfloat16
    qkv_w = np.asarray(qkv_w, dtype=np.float32).astype(bf16)
    proj_w = np.asarray(proj_w, dtype=np.float32).astype(bf16)
    pb = np.asarray(proj_b, dtype=np.float32).reshape(1, C)
    return [
        {
            "x_t": np.ascontiguousarray(
                np.asarray(x[b], dtype=np.float32).T).astype(bf16),
            "qkv_w": qkv_w,
            "proj_w": proj_w,
            "proj_b": pb,
        }
        for b in range(N_CORES)
    ]


def kernel(x, qkv_w, proj_w, proj_b):
    runner = _get_runner()
    results = runner(make_in_maps(x, qkv_w, proj_w, proj_b))
    return np.stack([results[b]["out"] for b in range(N_CORES)], axis=0)
